# revision 17
# baseline (speedup 1.0000x reference)
"""AssociativeEmbeddingLoss on 8 TRN2 NeuronCores.

Reference computes, per image b (C=1, G=128 boxes):
    tl[g] = pred[b, 0, ty[g], tx[g]],  br[g] = target[b, 0, by[g], bx[g]]
    me = (tl + br) / 2
    pull_b = sum((tl-me)^2 + (br-me)^2) / N            = sum((tl-br)^2) / (2N)
    push_b = sum_{i != j} relu(1 - |me_i - me_j|) / (N*(N-1))
    out = (0.25 * sum_b pull_b, 0.25 * sum_b push_b)

Strategy: data-parallel over batch, 8 images per core. Only the 2*G
gathered scalars per image are ever read from the big [B,1,H,W] inputs:
the device gathers them with indirect DMA using flat indices computed
on-device from the (int32-cast) match coords. The N x N push term is
built per image via a rank-8 broadcast matmul into PSUM, fused
abs/relu/accumulate on the scalar engine, and partition-summed with a
ones matmul. Each core emits its partial [pull_sum, push_sum]; the
host sums the 8 partial pairs (the unshard step).
"""

import numpy as np

import concourse.bacc as bacc
import concourse.bass as bass
import concourse.mybir as mybir
import concourse.tile as tile
from concourse.bass import IndirectOffsetOnAxis
from concourse.bass_utils import run_bass_kernel_spmd

B, C, H, W = 64, 1, 512, 512
G = 128                 # boxes per image; N = G*C = 128
N = G * C
NCORES = 8
BP = B // NCORES        # images per core
NPIX = BP * H * W       # flattened pred/target elements per core
PULL_W, PUSH_W = 0.25, 0.25

F32 = mybir.dt.float32
I32 = mybir.dt.int32
AF = mybir.ActivationFunctionType
ALU = mybir.AluOpType


def _build_nc():
    nc = bacc.Bacc(
        "TRN2",
        target_bir_lowering=False,
        debug=False,
        enable_asserts=False,
        num_devices=NCORES,
    )
    pred = nc.dram_tensor("pred", [NPIX, 1], F32, kind="ExternalInput")
    targ = nc.dram_tensor("target", [NPIX, 1], F32, kind="ExternalInput")
    match = nc.dram_tensor("match", [BP, G * 4], F32, kind="ExternalInput")
    out = nc.dram_tensor("out", [1, 2], F32, kind="ExternalOutput")

    with tile.TileContext(nc) as tc:
        _kernel_body(nc, tc, pred, targ, match, out)
    nc.compile()
    return nc


def _kernel_body(nc, tc, pred, targ, match, out):
    with (
        tc.tile_pool(name="sb", bufs=1) as sb,
        tc.tile_pool(name="ps", bufs=1, space="PSUM") as ps,
    ):
        # ---- constants ----
        ones = sb.tile([128, 1], F32, tag="ones")
        nc.gpsimd.memset(ones[:], 1.0)
        # identity for the [G, BP] -> [BP, G] PE transpose
        ident = sb.tile([G, G], F32, tag="ident")
        nc.gpsimd.memset(ident[:], 0.0)
        nc.gpsimd.affine_select(
            out=ident[:], in_=ident[:], compare_op=ALU.not_equal, fill=1.0,
            base=0, pattern=[[-1, G]], channel_multiplier=1,
        )
        # blk[k, b*G + i] = (k == b): selector weights for row-broadcast.
        # iota gives (col // G) - k; is_equal against 0 makes the band.
        blk_pre = sb.tile([BP, BP * G], F32, tag="blk_pre")
        nc.gpsimd.iota(
            blk_pre[:], pattern=[[1, BP], [0, G]], base=0, channel_multiplier=-1,
            allow_small_or_imprecise_dtypes=True,
        )
        blk = sb.tile([BP, BP * G], F32, tag="blk")
        nc.gpsimd.tensor_scalar(
            out=blk[:], in0=blk_pre[:], scalar1=0.0, scalar2=None, op0=ALU.is_equal,
        )
        # per-image row base, varying along free: base_row[:, b] = b * H
        base_row = sb.tile([G, BP], F32, tag="base_row")
        nc.gpsimd.iota(
            base_row[:], pattern=[[H, BP]], base=0, channel_multiplier=0,
            allow_small_or_imprecise_dtypes=True,
        )

        # ---- coords -> flat gather indices ([G, BP] layout; f32, exact) ----
        # match[b, g*4+c] viewed as [g partitions, (b, c) free]
        coords = sb.tile([G, BP * 4], F32, tag="coords")
        mr = match.ap().rearrange("b (g c) -> g b c", c=4)
        cr = coords[:].rearrange("g (b c) -> g b c", c=4)
        nc.gpsimd.dma_start(out=cr, in_=mr)
        tl_f = sb.tile([G, BP], F32, tag="tlf")
        br_f = sb.tile([G, BP], F32, tag="brf")
        tl_idx = sb.tile([G, BP], I32, tag="tlidx")
        br_idx = sb.tile([G, BP], I32, tag="bridx")
        # idx = (y + b*H) * W + x
        nc.vector.tensor_tensor(out=tl_f[:], in0=cr[:, :, 0], in1=base_row[:], op=ALU.add)
        nc.vector.tensor_scalar(
            out=tl_f[:], in0=tl_f[:], scalar1=float(W), scalar2=None, op0=ALU.mult,
        )
        nc.vector.tensor_tensor(out=tl_f[:], in0=tl_f[:], in1=cr[:, :, 1], op=ALU.add)
        nc.vector.tensor_copy(out=tl_idx[:], in_=tl_f[:])
        nc.vector.tensor_tensor(out=br_f[:], in0=cr[:, :, 2], in1=base_row[:], op=ALU.add)
        nc.vector.tensor_scalar(
            out=br_f[:], in0=br_f[:], scalar1=float(W), scalar2=None, op0=ALU.mult,
        )
        nc.vector.tensor_tensor(out=br_f[:], in0=br_f[:], in1=cr[:, :, 3], op=ALU.add)
        nc.vector.tensor_copy(out=br_idx[:], in_=br_f[:])

        # ---- gather tl/br embeddings: [G, BP], one column per image ----
        # HW indirect DMA gathers one row per partition from the offset in
        # that partition, so each image's 128 scalars are one DMA.
        tl = sb.tile([G, BP], F32, tag="tl")
        br = sb.tile([G, BP], F32, tag="br")
        for b in range(BP):
            nc.gpsimd.indirect_dma_start(
                out=tl[:, b : b + 1], out_offset=None, in_=pred.ap(),
                in_offset=IndirectOffsetOnAxis(ap=tl_idx[:, b : b + 1], axis=0),
            )
            nc.gpsimd.indirect_dma_start(
                out=br[:, b : b + 1], out_offset=None, in_=targ.ap(),
                in_offset=IndirectOffsetOnAxis(ap=br_idx[:, b : b + 1], axis=0),
            )

        # ---- pull: sum((tl-br)^2) over everything ----
        dsub = sb.tile([G, BP], F32, tag="dsub")
        nc.vector.tensor_sub(dsub[:], tl[:], br[:])
        sq = sb.tile([G, BP], F32, tag="sq")
        pull_part = sb.tile([G, 1], F32, tag="pull_part")
        nc.scalar.activation(out=sq[:], in_=dsub[:], func=AF.Square, accum_out=pull_part[:])

        # ---- push: me2 = tl + br = 2*me, in column layout [G, BP] ----
        me2c = sb.tile([G, BP], F32, tag="me2c")
        nc.vector.tensor_add(me2c[:], tl[:], br[:])
        # negme[i, b] = -me_b[i] (scale folds the /2): directly from me2c
        negme = sb.tile([G, BP], F32, tag="negme")
        nc.scalar.activation(out=negme[:], in_=me2c[:], func=AF.Copy, scale=-0.5)
        # row layout [BP, G] via PE transpose
        me2r_ps = ps.tile([BP, G], F32, tag="me2r_ps")
        nc.tensor.transpose(out=me2r_ps[:], in_=me2c[:], identity=ident[:])
        me2r = sb.tile([BP, G], F32, tag="me2r")
        nc.vector.tensor_copy(out=me2r[:], in_=me2r_ps[:])

        # row-broadcast: R[i, (b, j)] = me2[b, j] via rank-BP matmuls
        R0 = ps.tile([G, 512], F32, tag="r0")
        R1 = ps.tile([G, 512], F32, tag="r1")
        for b in range(BP):
            Rt = R0 if b < 4 else R1
            off = (b % 4) * G
            nc.tensor.matmul(
                out=Rt[:, off : off + G],
                lhsT=blk[:, b * G : (b + 1) * G],
                rhs=me2r[:],
                start=True, stop=True,
            )
        # absd[i, (b,j)] = |0.5*me2[b,j] - 0.5*me2[b,i]| = |me_j - me_i|
        absd = sb.tile([G, BP * G], F32, tag="absd")
        for b in range(BP):
            Rt = R0 if b < 4 else R1
            off = (b % 4) * G
            nc.scalar.activation(
                out=absd[:, b * G : (b + 1) * G], in_=Rt[:, off : off + G],
                func=AF.Abs, bias=negme[:, b : b + 1], scale=0.5,
            )
        # conf = relu(1 - absd); push_col[i] = sum_j conf (incl. diagonal 1s)
        conf = sb.tile([G, BP * G], F32, tag="conf")
        push_col = sb.tile([G, 1], F32, tag="push_col")
        nc.scalar.activation(
            out=conf[:], in_=absd[:], func=AF.Relu, bias=1.0, scale=-1.0,
            accum_out=push_col[:],
        )

        # ---- partition sums -> [1, 2] scalars ----
        fin = ps.tile([1, 2], F32, tag="fin")
        nc.tensor.matmul(
            out=fin[0:1, 0:1], lhsT=pull_part[:], rhs=ones[:, 0:1],
            start=True, stop=True,
        )
        nc.tensor.matmul(
            out=fin[0:1, 1:2], lhsT=push_col[:], rhs=ones[:, 0:1],
            start=True, stop=True,
        )
        # scale on the way out; subtract the BP*N diagonal relu(1)=1 terms
        c_pull = PULL_W / (2.0 * N)
        c_push = PUSH_W / (N * (N - 1))
        res = sb.tile([1, 2], F32, tag="res")
        nc.scalar.activation(out=res[0:1, 0:1], in_=fin[0:1, 0:1], func=AF.Copy, scale=c_pull)
        nc.scalar.activation(
            out=res[0:1, 1:2], in_=fin[0:1, 1:2], func=AF.Copy,
            scale=c_push, bias=-float(BP * N) * c_push,
        )
        nc.sync.dma_start(out=out.ap(), in_=res[:])


_NC_CACHE = None


def _get_nc():
    global _NC_CACHE
    if _NC_CACHE is None:
        _NC_CACHE = _build_nc()
    return _NC_CACHE


def make_in_maps(pred, target, match):
    pred = np.asarray(pred, dtype=np.float32)
    target = np.asarray(target, dtype=np.float32)
    match = np.asarray(match)
    in_maps = []
    for k in range(NCORES):
        sl = slice(k * BP, (k + 1) * BP)
        in_maps.append({
            "pred": np.ascontiguousarray(pred[sl]).reshape(NPIX, 1),
            "target": np.ascontiguousarray(target[sl]).reshape(NPIX, 1),
            "match": np.ascontiguousarray(match[sl]).astype(np.float32).reshape(BP, G * 4),
        })
    return in_maps


def kernel(pred, target, match, _trace=False):
    nc = _get_nc()
    in_maps = make_in_maps(pred, target, match)
    res = run_bass_kernel_spmd(nc, in_maps, core_ids=list(range(NCORES)), trace=_trace)
    total = np.zeros((1, 2), dtype=np.float64)
    for r in res.results:
        total += r["out"].astype(np.float64)
    out = (np.float32(total[0, 0]), np.float32(total[0, 1]))
    if _trace:
        return out, res
    return out


# revision 23
# speedup vs baseline: 1.2635x; 1.2635x over previous
"""AssociativeEmbeddingLoss on 8 TRN2 NeuronCores.

Reference, per image b (C=1, G=128 boxes):
    tl[g] = pred[b, 0, ty[g], tx[g]],  br[g] = target[b, 0, by[g], bx[g]]
    me = (tl + br) / 2
    pull_b = sum((tl-br)^2) / (2N)
    push_b = sum_{i != j} relu(1 - |me_i - me_j|) / (N*(N-1))
    out = (0.25 * sum_b pull_b, 0.25 * sum_b push_b)

Data-parallel over batch, 8 images per core. Only the 2*G*BP scalars the
loss touches are read from the [B,1,H,W] inputs: two `dma_gather` ops
(64-wide rows; row = flat_idx >> 6, all-int16 index math on DVE) fetch
them, a one-hot multiply+reduce extracts the in-row element
(flat & 63 == x & 63). The N x N push term uses a rank-8 broadcast
matmul into PSUM, fused |me_j - me_i| on the scalar engine
(bias = -me_i), and the identity relu(1-|d|) = 1 - min(|d|,1) so one
DVE min+accumulate finishes the reduction. Each core emits its partial
[pull_sum, min_sum]; the host combines the 8 pairs (the unshard step).

HW quirk: the dma_gather ucode reads index slot k from partition
16 + k%16 (the simulator reads partition k%16), so indices are written
duplicated to partitions 0..15 and 16..31.
"""

import numpy as np

import concourse.bacc as bacc
import concourse.bass as bass
import concourse.mybir as mybir
import concourse.tile as tile
from concourse import library_config
from concourse.bass_utils import run_bass_kernel_spmd

B, C, H, W = 64, 1, 512, 512
G = 128                 # boxes per image; N = G*C = 128
N = G * C
NCORES = 8
BP = B // NCORES        # images per core
NPIX = BP * H * W       # flattened pred/target elements per core
ES = 64                 # dma_gather element (row) size in f32
NROW = NPIX // ES       # 32768 rows, row index fits int16
NIDX = BP * G           # 1024 gather indices per tensor
PULL_W, PUSH_W = 0.25, 0.25

F32 = mybir.dt.float32
I16 = mybir.dt.int16
AF = mybir.ActivationFunctionType
ALU = mybir.AluOpType


def _build_nc():
    nc = bacc.Bacc(
        "TRN2",
        target_bir_lowering=False,
        debug=False,
        enable_asserts=False,
        num_devices=NCORES,
    )
    pred = nc.dram_tensor("pred", [NROW, ES], F32, kind="ExternalInput")
    targ = nc.dram_tensor("target", [NROW, ES], F32, kind="ExternalInput")
    match = nc.dram_tensor("match", [BP, G * 4], I16, kind="ExternalInput")
    bconst = nc.dram_tensor("bconst", [32, ES], I16, kind="ExternalInput")
    iota64 = nc.dram_tensor("iota64", [128, 2 * BP * ES], F32, kind="ExternalInput")
    ident = nc.dram_tensor("ident", [G, G], F32, kind="ExternalInput")
    blk = nc.dram_tensor("blk", [BP, BP * G], F32, kind="ExternalInput")
    out = nc.dram_tensor("out", [1, 2], F32, kind="ExternalOutput")

    with tile.TileContext(nc) as tc:
        _kernel_body(nc, tc, pred, targ, match, bconst, iota64, ident, blk, out)
    nc.compile()
    return nc


def _kernel_body(nc, tc, pred, targ, match, bconst, iota64, ident, blk, out):
    with (
        tc.tile_pool(name="sb", bufs=1) as sb,
        tc.tile_pool(name="ps", bufs=1, space="PSUM") as ps,
    ):
        # gpsimd only runs the gather library + the two gathers
        nc.gpsimd.load_library(library_config.mlp)

        ones = sb.tile([128, 1], F32, tag="ones")
        nc.vector.memset(ones[:], 1.0)

        # ---- constants from host ----
        bc = sb.tile([32, ES], I16, tag="bc")
        nc.sync.dma_start(out=bc[:], in_=bconst.ap())
        io64 = sb.tile([128, 2 * BP * ES], F32, tag="io64")
        nc.sync.dma_start(out=io64[:], in_=iota64.ap())
        idt = sb.tile([G, G], F32, tag="idt")
        nc.sync.dma_start(out=idt[:], in_=ident.ap())
        bk = sb.tile([BP, BP * G], F32, tag="bk")
        nc.sync.dma_start(out=bk[:], in_=blk.ap())

        # ---- coords for the row-index pipeline, [32, (b, q, c)] i16 ----
        # partition p holds g == q*16 + (p % 16); rows 16..31 duplicate 0..15
        # (HW reads them, sim reads 0..15).
        c32 = sb.tile([32, BP * (G // 16) * 4], I16, tag="c32")
        src16 = bass.AP(
            match.ap().tensor, 0,
            [[4, 16], [G * 4, BP], [64, G // 16], [1, 4]],
        )
        nc.sync.dma_start(
            out=c32[0:16, :].rearrange("p (b q c) -> p b q c", b=BP, q=G // 16, c=4),
            in_=src16,
        )
        nc.sync.dma_start(
            out=c32[16:32, :].rearrange("p (b q c) -> p b q c", b=BP, q=G // 16, c=4),
            in_=src16,
        )
        cr = c32[:].rearrange("p (b q c) -> p b q c", b=BP, q=G // 16, c=4)  # [32, BP, 8, 4]

        # row = (y << 3) + (b << 12) + (x >> 6), all int16, exact
        def rowidx(name, ysel, xsel):
            idx = sb.tile([128, NIDX // 16], I16, tag=name)
            nc.vector.memset(idx[:], 0)
            t = sb.tile([32, NIDX // 16], I16, tag=name + "_t")
            u = sb.tile([32, NIDX // 16], I16, tag=name + "_u")
            tv = t[:].rearrange("p (b q) -> p b q", b=BP, q=G // 16)
            uv = u[:].rearrange("p (b q) -> p b q", b=BP, q=G // 16)
            nc.vector.tensor_scalar(
                out=tv, in0=cr[:, :, :, ysel], scalar1=3, scalar2=None,
                op0=ALU.logical_shift_left,
            )
            nc.vector.tensor_tensor(out=t[:], in0=t[:], in1=bc[:], op=ALU.add)
            nc.vector.tensor_scalar(
                out=uv, in0=cr[:, :, :, xsel], scalar1=6, scalar2=None,
                op0=ALU.logical_shift_right,
            )
            nc.vector.tensor_tensor(out=idx[0:32, :], in0=t[:], in1=u[:], op=ALU.add)
            return idx

        tl_idx = rowidx("tlidx", 0, 1)
        br_idx = rowidx("bridx", 2, 3)

        # ---- gathers: rows64[:, 0:8, :] = tl rows, [:, 8:16, :] = br rows ----
        rows64 = sb.tile([128, 2 * BP, ES], F32, tag="rows64")
        nc.gpsimd.dma_gather(rows64[:, 0:BP, :], pred.ap(), tl_idx[:], NIDX, NIDX, ES)
        nc.gpsimd.dma_gather(rows64[:, BP : 2 * BP, :], targ.ap(), br_idx[:], NIDX, NIDX, ES)

        # ---- in-row position: rem = x & 63, in (s, b) order matching rows64 ----
        # coordsC: partition g, free (b, c)
        cC = sb.tile([128, BP * 4], I16, tag="cC")
        srcC = bass.AP(match.ap().tensor, 0, [[4, G], [G * 4, BP], [1, 4]])
        nc.sync.dma_start(out=cC[:].rearrange("g (b c) -> g b c", b=BP, c=4), in_=srcC)
        # x coords at c = 1 (tl) and c = 3 (br): view (g, s, b)
        xv = cC[:].rearrange("g (b c2 two) -> g c2 b two", b=BP, two=2)[:, :, :, 1]
        rem16 = sb.tile([128, 2 * BP], I16, tag="rem16")
        nc.vector.tensor_scalar(
            out=rem16[:].rearrange("g (s b) -> g s b", s=2, b=BP), in0=xv,
            scalar1=63, scalar2=None, op0=ALU.bitwise_and,
        )
        remf = sb.tile([128, 2 * BP], F32, tag="remf")
        nc.vector.tensor_copy(out=remf[:], in_=rem16[:])

        # ---- extract: tlbr[g, (s,b)] = rows64[g, (s,b), rem] ----
        oh = sb.tile([128, 2 * BP * ES], F32, tag="oh")
        nc.vector.tensor_tensor(
            out=oh[:].rearrange("p (a b) -> p a b", a=2 * BP, b=ES),
            in0=io64[:].rearrange("p (a b) -> p a b", a=2 * BP, b=ES),
            in1=remf[:, :, None].to_broadcast([128, 2 * BP, ES]),
            op=ALU.is_equal,
        )
        prod = sb.tile([128, 2 * BP * ES], F32, tag="prod")
        nc.vector.tensor_tensor(
            out=prod[:], in0=oh[:], in1=rows64[:].rearrange("p a b -> p (a b)"),
            op=ALU.mult,
        )
        tlbr = sb.tile([128, 2 * BP], F32, tag="tlbr")
        nc.vector.tensor_reduce(
            out=tlbr[:], in_=prod[:].rearrange("p (a b) -> p a b", a=2 * BP, b=ES),
            op=ALU.add, axis=mybir.AxisListType.X,
        )
        tl = tlbr[:, 0:BP]
        br = tlbr[:, BP : 2 * BP]

        # ---- pull: sum((tl-br)^2) ----
        dsub = sb.tile([128, BP], F32, tag="dsub")
        nc.vector.tensor_sub(dsub[:], tl, br)
        dscr = sb.tile([128, BP], F32, tag="dscr")
        pull_col = sb.tile([128, 1], F32, tag="pull_col")
        nc.vector.tensor_mul(dscr[:], dsub[:], dsub[:])
        nc.vector.tensor_reduce(
            out=pull_col[:], in_=dscr[:], op=ALU.add, axis=mybir.AxisListType.X,
        )

        # ---- push ----
        me2c = sb.tile([128, BP], F32, tag="me2c")
        nc.vector.tensor_add(me2c[:], tl, br)
        negme = sb.tile([128, BP], F32, tag="negme")   # -me_i
        nc.vector.tensor_scalar(
            out=negme[:], in0=me2c[:], scalar1=-0.5, scalar2=None, op0=ALU.mult,
        )
        me2r_ps = ps.tile([BP, G], F32, tag="me2r_ps")
        nc.tensor.transpose(out=me2r_ps[:], in_=me2c[:], identity=idt[:])
        me_r = sb.tile([BP, G], F32, tag="me_r")       # +me_j (row layout)
        nc.vector.tensor_scalar(
            out=me_r[:], in0=me2r_ps[:], scalar1=0.5, scalar2=None, op0=ALU.mult,
        )
        # R[i, (b, j)] = me[b, j]
        R0 = ps.tile([G, 512], F32, tag="r0")
        R1 = ps.tile([G, 512], F32, tag="r1")
        absd = sb.tile([G, BP * G], F32, tag="absd")
        for b in range(BP):
            Rt = R0 if b < 4 else R1
            off = (b % 4) * G
            nc.tensor.matmul(
                out=Rt[:, off : off + G], lhsT=bk[:, b * G : (b + 1) * G],
                rhs=me_r[:], start=True, stop=True,
            )
            nc.scalar.activation(
                out=absd[:, b * G : (b + 1) * G], in_=Rt[:, off : off + G],
                func=AF.Abs, bias=negme[:, b : b + 1], scale=1.0,
            )
        # relu(1-|d|) = 1 - min(|d|,1): accumulate min(|d|,1)
        mscr = sb.tile([G, BP * G], F32, tag="mscr")
        min_col = sb.tile([G, 1], F32, tag="min_col")
        nc.vector.tensor_scalar(
            out=mscr[:], in0=absd[:], scalar1=1.0, scalar2=0.0,
            op0=ALU.min, op1=ALU.add, accum_out=min_col[:],
        )

        # ---- partition sums -> [1, 2] ----
        fin = ps.tile([1, 2], F32, tag="fin")
        nc.tensor.matmul(out=fin[0:1, 0:1], lhsT=pull_col[:], rhs=ones[:, 0:1],
                         start=True, stop=True)
        nc.tensor.matmul(out=fin[0:1, 1:2], lhsT=min_col[:], rhs=ones[:, 0:1],
                         start=True, stop=True)
        # pull_all_part = c_pull * S; push: conf_sum = BP*N*N - minsum,
        # minus BP*N diagonal -> (BP*N*(N-1) - minsum) * c_push
        c_pull = PULL_W / (2.0 * N)
        c_push = PUSH_W / (N * (N - 1))
        res = sb.tile([1, 2], F32, tag="res")
        nc.scalar.activation(out=res[0:1, 0:1], in_=fin[0:1, 0:1], func=AF.Copy,
                             scale=c_pull)
        nc.scalar.activation(out=res[0:1, 1:2], in_=fin[0:1, 1:2], func=AF.Copy,
                             scale=-c_push, bias=float(BP * N * (N - 1)) * c_push)
        nc.sync.dma_start(out=out.ap(), in_=res[:])


_NC_CACHE = None


def _get_nc():
    global _NC_CACHE
    if _NC_CACHE is None:
        _NC_CACHE = _build_nc()
    return _NC_CACHE


def _consts():
    bvals = ((np.arange(ES) // (G // 16)) << 12).astype(np.int16)
    bconst = np.broadcast_to(bvals, (32, ES)).copy()
    iota64 = np.broadcast_to(
        np.tile(np.arange(ES, dtype=np.float32), 2 * BP), (128, 2 * BP * ES)
    ).copy()
    ident = np.eye(G, dtype=np.float32)
    blk = np.zeros((BP, BP * G), dtype=np.float32)
    for b in range(BP):
        blk[b, b * G : (b + 1) * G] = 1.0
    return bconst, iota64, ident, blk


def make_in_maps(pred, target, match):
    pred = np.asarray(pred, dtype=np.float32)
    target = np.asarray(target, dtype=np.float32)
    match = np.asarray(match)
    bconst, iota64, ident, blk = _consts()
    in_maps = []
    for k in range(NCORES):
        sl = slice(k * BP, (k + 1) * BP)
        in_maps.append({
            "pred": np.ascontiguousarray(pred[sl]).reshape(NROW, ES),
            "target": np.ascontiguousarray(target[sl]).reshape(NROW, ES),
            "match": np.ascontiguousarray(match[sl]).astype(np.int16).reshape(BP, G * 4),
            "bconst": bconst, "iota64": iota64, "ident": ident, "blk": blk,
        })
    return in_maps


def kernel(pred, target, match, _trace=False):
    nc = _get_nc()
    in_maps = make_in_maps(pred, target, match)
    res = run_bass_kernel_spmd(nc, in_maps, core_ids=list(range(NCORES)), trace=_trace)
    total = np.zeros((1, 2), dtype=np.float64)
    for r in res.results:
        total += r["out"].astype(np.float64)
    out = (np.float32(total[0, 0]), np.float32(total[0, 1]))
    if _trace:
        return out, res
    return out


# revision 27
# speedup vs baseline: 1.8003x; 1.4248x over previous
"""AssociativeEmbeddingLoss on 8 TRN2 NeuronCores.

Reference, per image b (C=1, G=128 boxes):
    tl[g] = pred[b, 0, ty[g], tx[g]],  br[g] = target[b, 0, by[g], bx[g]]
    me = (tl + br) / 2
    pull_b = sum((tl-br)^2) / (2N)
    push_b = sum_{i != j} relu(1 - |me_i - me_j|) / (N*(N-1))
    out = (0.25 * sum_b pull_b, 0.25 * sum_b push_b)

Data-parallel over batch, 8 images per core. Only the 2*G*BP scalars the
loss touches are read from the big inputs, via 16 indirect DMAs (one
[128,1] column per image/tensor; the Q7 descriptor cost ~10ns/element is
the hard floor either way). Flat gather indices are computed on DVE in
f32 (exact below 2^24). Per-image compute (me, transpose, row-broadcast
matmul, |me_j - me_i|, and relu(1-|d|) = 1 - min(|d|,1) min+accumulate)
is pipelined behind the remaining gathers, so the post-gather tail is a
couple of tiny reductions. Each core emits its partial
[pull_sum, min_sum]; the host combines the 8 pairs (the unshard step).
"""

import numpy as np

import concourse.bacc as bacc
import concourse.bass as bass
import concourse.mybir as mybir
import concourse.tile as tile
from concourse.bass import IndirectOffsetOnAxis
from concourse.bass_utils import run_bass_kernel_spmd

B, C, H, W = 64, 1, 512, 512
G = 128                 # boxes per image; N = G*C = 128
N = G * C
NCORES = 8
BP = B // NCORES        # images per core
NPIX = BP * H * W
PULL_W, PUSH_W = 0.25, 0.25

F32 = mybir.dt.float32
I32 = mybir.dt.int32
AF = mybir.ActivationFunctionType
ALU = mybir.AluOpType

# |d| via DVE tensor_scalar op1=abs_max (else scalar-engine Abs activation)
USE_DVE_ABS = False


def _build_nc():
    nc = bacc.Bacc(
        "TRN2",
        target_bir_lowering=False,
        debug=False,
        enable_asserts=False,
        num_devices=NCORES,
    )
    pred = nc.dram_tensor("pred", [NPIX, 1], F32, kind="ExternalInput")
    targ = nc.dram_tensor("target", [NPIX, 1], F32, kind="ExternalInput")
    match = nc.dram_tensor("match", [BP, G * 4], F32, kind="ExternalInput")
    # consts: [:, 0:128] identity, [:, 128:136] base_row (b*H), [:, 136] ones
    consts = nc.dram_tensor("consts", [G, 2 * G + BP + 1], F32, kind="ExternalInput")
    out = nc.dram_tensor("out", [1, 2], F32, kind="ExternalOutput")

    with tile.TileContext(nc) as tc:
        _kernel_body(nc, tc, pred, targ, match, consts, out)
    nc.compile()
    return nc


def _kernel_body(nc, tc, pred, targ, match, consts, out):
    with (
        tc.tile_pool(name="sb", bufs=1) as sb,
        tc.tile_pool(name="ps", bufs=1, space="PSUM") as ps,
        tc.tile_pool(name="psr", bufs=2, space="PSUM") as psr,
    ):
        # ---- coords first: [128, (b, c)] f32, partition = g ----
        cC = sb.tile([G, BP * 4], F32, tag="cC")
        srcC = bass.AP(match.ap().tensor, 0, [[4, G], [G * 4, BP], [1, 4]])
        nc.sync.dma_start(out=cC[:].rearrange("g (b c) -> g b c", b=BP, c=4), in_=srcC)
        cr = cC[:].rearrange("g (b c) -> g b c", b=BP, c=4)

        ct = sb.tile([G, 2 * G + BP + 1], F32, tag="ct")
        nc.sync.dma_start(out=ct[:], in_=consts.ap())
        ident = ct[:, 0:G]
        base_row = ct[:, G : G + BP]          # [128, BP] value b*H
        ones = ct[:, G + BP : G + BP + 1]     # [128, 1]
        ones_row = ct[0:1, G + BP + 1 : G + BP + 1 + G]   # [1, 128]

        # ---- flat indices: idx = (y + b*H) * W + x  (f32 exact) ----
        def flatidx(name, ysel, xsel):
            f = sb.tile([G, BP], F32, tag=name + "_f")
            i = sb.tile([G, BP], I32, tag=name)
            nc.vector.tensor_tensor(out=f[:], in0=cr[:, :, ysel], in1=base_row, op=ALU.add)
            nc.vector.tensor_scalar(
                out=f[:], in0=f[:], scalar1=float(W), scalar2=None, op0=ALU.mult,
            )
            nc.vector.tensor_tensor(out=f[:], in0=f[:], in1=cr[:, :, xsel], op=ALU.add)
            nc.vector.tensor_copy(out=i[:], in_=f[:])
            return i

        tl_idx = flatidx("tlidx", 0, 1)
        br_idx = flatidx("bridx", 2, 3)

        # ---- gathers + per-image pipeline ----
        tl = sb.tile([G, BP], F32, tag="tl")
        br = sb.tile([G, BP], F32, tag="br")
        me2c = sb.tile([G, BP], F32, tag="me2c")
        negme = sb.tile([G, BP], F32, tag="negme")
        dsub = sb.tile([G, BP], F32, tag="dsub")
        min_cols = sb.tile([G, BP], F32, tag="min_cols")
        absd = None
        if not USE_DVE_ABS:
            absd = sb.tile([G, BP * G], F32, tag="absd")

        for b in range(BP):
            cs = slice(b, b + 1)
            nc.gpsimd.indirect_dma_start(
                out=tl[:, cs], out_offset=None, in_=pred.ap(),
                in_offset=IndirectOffsetOnAxis(ap=tl_idx[:, cs], axis=0),
            )
            nc.gpsimd.indirect_dma_start(
                out=br[:, cs], out_offset=None, in_=targ.ap(),
                in_offset=IndirectOffsetOnAxis(ap=br_idx[:, cs], axis=0),
            )
            # per-image compute, overlapping the remaining gathers
            nc.vector.tensor_sub(dsub[:, cs], tl[:, cs], br[:, cs])
            nc.vector.tensor_add(me2c[:, cs], tl[:, cs], br[:, cs])
            nc.vector.tensor_scalar(
                out=negme[:, cs], in0=me2c[:, cs], scalar1=-0.5, scalar2=None,
                op0=ALU.mult,
            )
            # me row: transpose the column, scale 0.5 on the copy out of PSUM
            rowp = psr.tile([1, G], F32, tag="rowp")
            nc.tensor.transpose(out=rowp[:], in_=me2c[:, cs], identity=ident)
            merow = sb.tile([1, G], F32, tag=f"merow{b % 2}")
            nc.vector.tensor_scalar(
                out=merow[:], in0=rowp[:], scalar1=0.5, scalar2=None, op0=ALU.mult,
            )
            # R[i, j] = me[b, j]
            Rp = psr.tile([G, G], F32, tag="Rp")
            nc.tensor.matmul(
                out=Rp[:], lhsT=ones_row, rhs=merow[:], start=True, stop=True,
            )
            if USE_DVE_ABS:
                # |R - me_i| then min(.,1), accumulated along j
                ad = sb.tile([G, G], F32, tag=f"ad{b % 2}")
                nc.vector.tensor_scalar(
                    out=ad[:], in0=Rp[:], scalar1=negme[:, cs], scalar2=0.0,
                    op0=ALU.add, op1=ALU.abs_max,
                )
                nc.vector.tensor_scalar(
                    out=ad[:], in0=ad[:], scalar1=1.0, scalar2=0.0,
                    op0=ALU.min, op1=ALU.add, accum_out=min_cols[:, cs],
                )
            else:
                nc.scalar.activation(
                    out=absd[:, b * G : (b + 1) * G], in_=Rp[:],
                    func=AF.Abs, bias=negme[:, cs], scale=1.0,
                )
                nc.vector.tensor_scalar(
                    out=absd[:, b * G : (b + 1) * G],
                    in0=absd[:, b * G : (b + 1) * G], scalar1=1.0, scalar2=0.0,
                    op0=ALU.min, op1=ALU.add, accum_out=min_cols[:, cs],
                )

        # ---- tail reductions ----
        sq = sb.tile([G, BP], F32, tag="sq")
        nc.vector.tensor_mul(sq[:], dsub[:], dsub[:])
        pull_col = sb.tile([G, 1], F32, tag="pull_col")
        nc.vector.tensor_reduce(
            out=pull_col[:], in_=sq[:], op=ALU.add, axis=mybir.AxisListType.X,
        )
        min_col = sb.tile([G, 1], F32, tag="min_col")
        nc.vector.tensor_reduce(
            out=min_col[:], in_=min_cols[:], op=ALU.add, axis=mybir.AxisListType.X,
        )
        fin = ps.tile([1, 2], F32, tag="fin")
        nc.tensor.matmul(out=fin[0:1, 0:1], lhsT=pull_col[:], rhs=ones,
                         start=True, stop=True)
        nc.tensor.matmul(out=fin[0:1, 1:2], lhsT=min_col[:], rhs=ones,
                         start=True, stop=True)
        # pull = c_pull * S; push = (BP*N*(N-1) - minsum) * c_push
        c_pull = PULL_W / (2.0 * N)
        c_push = PUSH_W / (N * (N - 1))
        res = sb.tile([1, 2], F32, tag="res")
        nc.scalar.activation(out=res[0:1, 0:1], in_=fin[0:1, 0:1], func=AF.Copy,
                             scale=c_pull)
        nc.scalar.activation(out=res[0:1, 1:2], in_=fin[0:1, 1:2], func=AF.Copy,
                             scale=-c_push, bias=float(BP * N * (N - 1)) * c_push)
        nc.sync.dma_start(out=out.ap(), in_=res[:])


_NC_CACHE = None


def _get_nc():
    global _NC_CACHE
    if _NC_CACHE is None:
        _NC_CACHE = _build_nc()
    return _NC_CACHE


def _consts():
    c = np.zeros((G, 2 * G + BP + 1), dtype=np.float32)
    c[:, 0:G] = np.eye(G, dtype=np.float32)
    c[:, G : G + BP] = (np.arange(BP, dtype=np.float32) * H)[None, :]
    c[:, G + BP] = 1.0
    c[0, G + BP + 1 :] = 1.0
    return c


def make_in_maps(pred, target, match):
    pred = np.asarray(pred, dtype=np.float32)
    target = np.asarray(target, dtype=np.float32)
    match = np.asarray(match)
    consts = _consts()
    in_maps = []
    for k in range(NCORES):
        sl = slice(k * BP, (k + 1) * BP)
        in_maps.append({
            "pred": np.ascontiguousarray(pred[sl]).reshape(NPIX, 1),
            "target": np.ascontiguousarray(target[sl]).reshape(NPIX, 1),
            "match": np.ascontiguousarray(match[sl]).astype(np.float32).reshape(BP, G * 4),
            "consts": consts,
        })
    return in_maps


def kernel(pred, target, match, _trace=False):
    nc = _get_nc()
    in_maps = make_in_maps(pred, target, match)
    res = run_bass_kernel_spmd(nc, in_maps, core_ids=list(range(NCORES)), trace=_trace)
    total = np.zeros((1, 2), dtype=np.float64)
    for r in res.results:
        total += r["out"].astype(np.float64)
    out = (np.float32(total[0, 0]), np.float32(total[0, 1]))
    if _trace:
        return out, res
    return out
